# revision 35
# baseline (speedup 1.0000x reference)
"""AttentionHead kernel for 8 Trainium2 NeuronCores — fp8/DoubleRow design.

Problem (per sample, B=4): x:[256,64,64] -> q/k/v 1x1-conv projections
(+positional encoding on q,k), S = q^T k / 8, softmax over the QUERY axis,
out = attn @ v, then 1x1-conv MLP with Mish + residual.

Sharding: 2 cores per sample, split over the query axis i (2048 queries each).
softmax normalizes over i, so the per-key denominator den[j] = sum_i exp(S[i,j])
needs one tiny cross-pair AllReduce (3 chunks, latency hidden behind the
exp stream / attn@v); den folds into v, everything else is local.

Design notes (vs the 194us bf16 baseline):
- The critical path is ScalarE's exp stream: 32 x [128,2048] tiles at ~2.1us
  each, running gapless on fully double-buffered PSUM (two alternating 4-bank
  pools). S matmuls (bf16, ~0.43us/tile at the 1.2GHz mid p-state) hide
  underneath, as do the v-projection "mega tiles" (8 j-tiles of fp8-DoubleRow
  matmul + one DVE copy) that borrow stream PSUM slots.
- attn@v, q/k/v projections and both MLP matmuls run fp8e4 DoubleRow (two
  128-deep k-tiles per matmul at 0.5 cycles/row), pairing adjacent j-tiles /
  channel halves via 3D APs over the natural layouts — no repacking.
  Measured end-to-end rel err ~7e-4 (residual + MLP attenuate the
  attention-path quantization).
- attn is stored as fp8 exp(S/8 - 2); the -2 keeps e4m3 in range and cancels
  in the softmax. ScalarE writes it directly from PSUM with the denominator
  accumulated for free. v is pre-scaled by 32 (folded into Wv/bv on host) so
  vts = v*32/den fits fp8; the attn@v PSUM->SBUF copy divides by 32.
- Mish: no Mish/Softplus LUT exists on this stack, so h = (h1)*tanh(sp),
  sp = Ln(1 + e^{h1}) — Exp and Ln live in the same activation table set as
  the attention exps (zero switches); the Tanh batch costs the single table
  switch of the whole kernel. GpSimd is avoided for elementwise work (its
  software ops measure ~12ns/elem) and cannot touch PSUM.
- Input DMAs: the 4MB fp32 residual (xf) is deferred until mid-stream so the
  q/k-path tensors own the DMA engines at the start; a 1-element Exp preloads
  the activation table during the initial DMA wait.
"""

import numpy as np
import ml_dtypes

import concourse.bass as bass
import concourse.bacc as bacc
import concourse.mybir as mybir
import concourse.tile as tile
from concourse.tile_rust import add_dep_helper

BF16 = mybir.dt.bfloat16
F8 = mybir.dt.float8e4
F32 = mybir.dt.float32
AF = mybir.ActivationFunctionType
OP = mybir.AluOpType
DR = mybir.MatmulPerfMode.DoubleRow
bf16 = ml_dtypes.bfloat16
f8 = ml_dtypes.float8_e4m3

B, C, H, W = 4, 256, 64, 64
N = H * W            # 4096 pixels
QK = 64
IS = N // 2          # 2048 queries per core
NJT = N // 128       # 32 key tiles
NIB = IS // 512      # 4 i-blocks
NBOOT = 2            # S tiles computed during the projection boot
CH_A, CH_B = 16, 28  # den chunk boundaries: [0,16) [16,28) [28,32)
VS = 32.0            # v scale (folded into Wv/bv on host)
EXP_BIAS = -2.0
N_CORES = 8
REPLICA_GROUPS = [[0, 1], [2, 3], [4, 5], [6, 7]]


def build_program(n_cores: int = N_CORES) -> bass.Bass:
    nc = bacc.Bacc(
        "TRN2",
        target_bir_lowering=False,
        debug=False,
        enable_asserts=False,
        num_devices=n_cores,
    )

    # Per-core inputs. xq/xb hold the two 128-row channel halves as dim 1
    # (the DoubleRow pair axis for the channel contraction).
    xq_d = nc.dram_tensor("xq", [128, 2, IS], F8, kind="ExternalInput").ap()
    xb_d = nc.dram_tensor("xb", [128, 2, N], F8, kind="ExternalInput").ap()
    xf_d = nc.dram_tensor("xf", [128, 2 * IS], F32, kind="ExternalInput").ap()
    pe1q_d = nc.dram_tensor("pe1q", [QK, IS], BF16, kind="ExternalInput").ap()
    # Shared weights (same on all cores).
    pe1_d = nc.dram_tensor("pe1", [QK, N], BF16, kind="ExternalInput").ap()
    wqk_d = nc.dram_tensor("wqk", [128, 2, 128], F8, kind="ExternalInput").ap()
    # wmlp = wvt*VS | w1t | w2t along the last axis
    wmlp_d = nc.dram_tensor("wmlp", [128, 2, 768], F8, kind="ExternalInput").ap()
    bvb_d = nc.dram_tensor("bvb", [128, 8, 256], BF16, kind="ExternalInput").ap()
    bcols_d = nc.dram_tensor("bcols", [128, 4], F32, kind="ExternalInput").ap()

    y_d = nc.dram_tensor("y", [C, IS], F32, kind="ExternalOutput").ap()

    with tile.TileContext(nc) as tc:
        with (
            tc.tile_pool(name="const", bufs=1) as cpool,
            tc.tile_pool(name="qk", bufs=1) as qkpool,
            tc.tile_pool(name="den", bufs=1) as denpool,
            tc.tile_pool(name="io", bufs=1) as iopool,
            tc.tile_pool(name="dram", bufs=1, space="DRAM") as dram,
        ):
            ebias_sb = cpool.tile([128, 1], F32)
            nc.gpsimd.memset(ebias_sb[:], EXP_BIAS)
            # Preload the Exp activation table while input DMAs run.
            warm_sb = cpool.tile([128, 1], BF16)
            nc.scalar.activation(warm_sb[:], ebias_sb[:], AF.Exp)

            # Critical-path loads, all contiguous per partition: wqk, xq,
            # pe1q, then xb ordered so k block 0 (j 0..2047) lands first.
            wqk_sb = cpool.tile([128, 2, 128], F8)
            nc.sync.dma_start(wqk_sb[:], wqk_d[:])
            xq_sb = iopool.tile([128, 2, IS], F8)
            for kt in range(2):
                nc.sync.dma_start(xq_sb[:, kt, :], xq_d[:, kt, :])
            pe1q_sb = iopool.tile([QK, IS], BF16)
            nc.scalar.dma_start(pe1q_sb[:], pe1q_d[:])
            xb_sb = iopool.tile([128, 2, N], F8)
            for hh in range(2):
                for kt in range(2):
                    nc.sync.dma_start(xb_sb[:, kt, bass.ts(hh, N // 2)],
                                      xb_d[:, kt, bass.ts(hh, N // 2)])
            pe1_sb = iopool.tile([QK, N], BF16)
            pe1_is = []
            for hh in range(2):
                pe1_is.append(
                    nc.scalar.dma_start(pe1_sb[:, bass.ts(hh, N // 2)],
                                        pe1_d[:, bass.ts(hh, N // 2)]))

            wmlp_sb = cpool.tile([128, 2, 768], F8)
            bvb_sb = cpool.tile([128, 8, 256], BF16)
            bcols_sb = cpool.tile([128, 4], F32)
            nc.sync.dma_start(wmlp_sb[:], wmlp_d[:])
            nc.sync.dma_start(bvb_sb[:], bvb_d[:])
            nc.sync.dma_start(bcols_sb[:], bcols_d[:])
            xf_sb = iopool.tile([128, 2 * IS], F32)  # DMA deferred, see below

            wvt = wmlp_sb[:, :, 0:256]
            w1t = wmlp_sb[:, :, 256:512]
            w2t = wmlp_sb[:, :, 512:768]
            b1c = bcols_sb[:, 0:2]
            b2c = bcols_sb[:, 2:4]

            q_sb = qkpool.tile([QK, IS], BF16)
            k_sb = qkpool.tile([QK, N], BF16)
            vtpool = tc.alloc_tile_pool(name="vt", bufs=1, side="right")
            vt_sb = vtpool.tile([128, NJT, 256], F32)
            vts_sb = qkpool.tile([128, NJT, 256], F8)
            den_sb = denpool.tile([128, NJT], F32)
            den_h = denpool.tile([128, 2], F32)
            dsum_sb = denpool.tile([128, NJT], F32)
            rden_sb = denpool.tile([128, NJT], F32)

            with tc.tile_pool(name="attn", bufs=1) as apool:
                attn_sb = apool.tile([128, NJT, IS], F8)   # 8 MiB

                with tc.tile_pool(name="psB", bufs=1, space="PSUM") as psB:
                    exp_is = {}

                    def s_tile(t, pool):
                        ps = pool.tile([128, IS], F32, name="pss")
                        for ib in range(NIB):
                            nc.tensor.matmul(ps[:, bass.ts(ib, 512)],
                                             k_sb[:, bass.ts(t, 128)],
                                             q_sb[:, bass.ts(ib, 512)],
                                             start=True, stop=True)
                        exp_is[t] = nc.scalar.activation(
                            attn_sb[:, t, :], ps[:], AF.Exp, scale=0.125,
                            bias=ebias_sb[:], accum_out=den_sb[:, t:t + 1])

                    def v_mega(vm, pool):
                        ps = pool.tile([128, 4, 256], F32, name="pss")
                        for g in range(4):
                            jt = 4 * vm + g
                            nc.tensor.matmul(ps[:, g, :],
                                             xb_sb[:, :, bass.ts(jt, 128)],
                                             wvt, start=True, stop=True,
                                             perf_mode=DR)
                        nc.vector.tensor_add(vt_sb[:, 4 * vm:4 * vm + 4, :],
                                             ps[:], bvb_sb[:, 0:4, :])

                    # ---- boot: q/k projections + S tiles 0..2 ----
                    with (
                        tc.tile_pool(name="psQ", bufs=2, space="PSUM") as psQ,
                        tc.tile_pool(name="psK", bufs=2, space="PSUM") as psK,
                    ):
                        def q_proj(ib):
                            sl = bass.ts(ib, 512)
                            ps = psQ.tile([QK, 512], F32, name="psq")
                            nc.tensor.matmul(ps[:], wqk_sb[:, :, 0:QK],
                                             xq_sb[:, :, sl],
                                             start=True, stop=True,
                                             perf_mode=DR)
                            nc.vector.tensor_add(q_sb[:, sl], ps[:],
                                                 pe1q_sb[:, sl])

                        def k_proj(jb):
                            sl = bass.ts(jb, 512)
                            ps = psK.tile([QK, 512], F32, name="psk")
                            nc.tensor.matmul(ps[:], wqk_sb[:, :, QK:2 * QK],
                                             xb_sb[:, :, sl],
                                             start=True, stop=True,
                                             perf_mode=DR)
                            nc.vector.tensor_add(k_sb[:, sl], ps[:],
                                                 pe1_sb[:, sl])

                        # tile 0 in two i-halves so the exp stream starts as
                        # soon as q ib0/ib1 + the first k block are in
                        q_proj(0)
                        q_proj(1)
                        k_proj(0)
                        ps0 = psB.tile([128, IS], F32, name="pss")
                        for ib in range(2):
                            nc.tensor.matmul(ps0[:, bass.ts(ib, 512)],
                                             k_sb[:, 0:128],
                                             q_sb[:, bass.ts(ib, 512)],
                                             start=True, stop=True)
                        e0a = nc.scalar.activation(
                            attn_sb[:, 0, 0:1024], ps0[:, 0:1024],
                            AF.Exp, scale=0.125, bias=ebias_sb[:],
                            accum_out=den_h[:, 0:1])
                        q_proj(2)
                        q_proj(3)
                        for ib in range(2, 4):
                            nc.tensor.matmul(ps0[:, bass.ts(ib, 512)],
                                             k_sb[:, 0:128],
                                             q_sb[:, bass.ts(ib, 512)],
                                             start=True, stop=True)
                        nc.scalar.activation(
                            attn_sb[:, 0, 1024:2048], ps0[:, 1024:2048],
                            AF.Exp, scale=0.125, bias=ebias_sb[:],
                            accum_out=den_h[:, 1:2])
                        k_proj(1)
                        s_tile(1, psB)
                        for jb in range(2, 8):
                            k_proj(jb)
                        nc.vector.tensor_add(den_sb[:, 0:1], den_h[:, 0:1],
                                             den_h[:, 1:2])

                    # ---- den AllReduce within the core pair ----
                    def exchange(lo, hi, tag):
                        w = hi - lo
                        den_in = dram.tile([128, w], F32, name=f"den_in{tag}")
                        den_out = dram.tile([128, w], F32, name=f"den_out{tag}")
                        nc.sync.dma_start(den_in[:], den_sb[:, lo:hi])
                        nc.gpsimd.collective_compute(
                            "AllReduce", OP.add,
                            replica_groups=REPLICA_GROUPS,
                            ins=[den_in.opt()], outs=[den_out.opt()],
                        )
                        nc.sync.dma_start(dsum_sb[:, lo:hi], den_out[:])
                        nc.vector.reciprocal(rden_sb[:, lo:hi],
                                             dsum_sb[:, lo:hi])
                        for jt in range(lo, hi):
                            nc.vector.tensor_scalar_mul(vts_sb[:, jt, :],
                                                        vt_sb[:, jt, :],
                                                        rden_sb[:, jt:jt + 1])

                    # ---- main stream: S tiles 3..31 ping-ponging between the
                    # boot pool and a second 4-bank pool; v-projection mega
                    # tiles borrow stream slots ----
                    VM_AFTER = {4: 0, 7: 1, 10: 2, 13: 3, 16: 4, 19: 5, 22: 6, 25: 7}
                    with tc.tile_pool(name="psS", bufs=1, space="PSUM") as psS:
                        pools = [psB, psS]
                        pi = 0
                        for t in range(NBOOT, NJT):
                            s_tile(t, pools[pi % 2]); pi += 1
                            if t in VM_AFTER:
                                # reuse the slot of the tile just exp'd; the
                                # S-tile alternation parity is unaffected
                                v_mega(VM_AFTER[t], pools[(pi - 1) % 2])
                            if t == CH_A - 1:
                                exchange(0, CH_A, "A")
                                for hh in range(4):
                                    nc.sync.dma_start(
                                        xf_sb[:, bass.ts(hh, IS // 2)],
                                        xf_d[:, bass.ts(hh, IS // 2)])
                            elif t == CH_B - 1:
                                exchange(CH_A, CH_B, "B")
                        exchange(CH_B, NJT, "C")

                # ---- attn@v + MLP, [128,1024] grain (ih halves) ----
                with (
                    tc.tile_pool(name="oh", bufs=1) as ohpool,
                    tc.tile_pool(name="mtmp", bufs=2) as mpool,
                    tc.tile_pool(name="y", bufs=2) as ypool,
                    tc.tile_pool(name="psO", bufs=1, space="PSUM") as psO,
                ):
                    out_sb = ohpool.tile([128, 2, IS], F8)
                    h_sb = ohpool.tile([128, 2, IS], F8)
                    pso = {}
                    for mt in range(2):
                        for ih in range(2):
                            pso[mt, ih] = psO.tile([128, 1024], F32,
                                                   name=f"pso{mt}{ih}")

                    def av_pairs(mt, ih, plo, phi):
                        for p in range(plo, phi):
                            for q2 in range(2):
                                nc.tensor.matmul(
                                    pso[mt, ih][:, bass.ts(q2, 512)],
                                    vts_sb[:, 2 * p:2 * p + 2,
                                           bass.ts(mt, 128)],
                                    attn_sb[:, 2 * p:2 * p + 2,
                                            ih * 1024 + q2 * 512:
                                            ih * 1024 + (q2 + 1) * 512],
                                    start=(p == 0), stop=(p == NJT // 2 - 1),
                                    perf_mode=DR, skip_group_check=True)

                    sp_ts = {}
                    pre_tanh = []

                    t_ts = {}
                    exp_mis = []

                    def mlp_front(ih):
                        # out copies (both mt halves), then W1 and the mish
                        # exp for the two [128,1024] blocks of this ih (runs
                        # while later AV blocks matmul); the Ln/Tanh batches
                        # come after all fronts
                        sl = bass.ts(ih, 1024)
                        for mt in range(2):
                            ci = nc.scalar.activation(out_sb[:, mt, sl],
                                                      pso[mt, ih][:], AF.Copy,
                                                      scale=1.0 / VS)
                            pre_tanh.append(ci)
                        for mt in range(2):
                            ps = pso[mt, ih]
                            for q2 in range(2):
                                nc.tensor.matmul(
                                    ps[:, bass.ts(q2, 512)],
                                    w1t[:, :, bass.ts(mt, 128)],
                                    out_sb[:, :, ih * 1024 + q2 * 512:
                                           ih * 1024 + (q2 + 1) * 512],
                                    start=True, stop=True, perf_mode=DR,
                                    skip_group_check=True)
                            t_t = mpool.tile([128, 1024], BF16,
                                             name=f"mt{mt}{ih}", bufs=1)
                            ti = nc.scalar.activation(t_t[:], ps[:], AF.Exp,
                                                      bias=b1c[:, mt:mt + 1])
                            pre_tanh.append(ti)
                            exp_mis.append(ti)
                            t_ts[mt, ih] = t_t

                    # W1's out_sb input needs both mt halves of its ih, so
                    # blocks complete in (ih-major, mt-minor) order; each
                    # block's MLP front starts while the next blocks' attn@v
                    # matmuls keep the PE busy
                    AB, CE = CH_B // 2, NJT // 2
                    av_pairs(0, 0, 0, AB)
                    av_pairs(1, 0, 0, AB)
                    av_pairs(0, 1, 0, AB)
                    av_pairs(0, 0, AB, CE)
                    av_pairs(1, 0, AB, CE)
                    mlp_front(0)
                    av_pairs(1, 1, 0, AB)
                    av_pairs(0, 1, AB, CE)
                    av_pairs(1, 1, AB, CE)
                    mlp_front(1)

                    # ln batch (all exps first, one load), then the tanh
                    # batch (second load)
                    for ih in range(2):
                        for mt in range(2):
                            sp_t = mpool.tile([128, 1024], BF16,
                                              name=f"msp{mt}{ih}", bufs=1)
                            li = nc.scalar.activation(sp_t[:],
                                                      t_ts[mt, ih][:],
                                                      AF.Ln, bias=1.0)
                            for e in exp_mis:
                                add_dep_helper(li.ins, e.ins, sync=False,
                                               reason="batch act tables")
                            pre_tanh.append(li)
                            sp_ts[mt, ih] = sp_t

                    # tanh batch (the kernel's one table switch); pin every
                    # tanh after every exp/ln/copy so the scheduler can't
                    # ping-pong the act tables
                    th_ts = {}
                    for ih in range(2):
                        for mt in range(2):
                            th_t = mpool.tile([128, 1024], BF16,
                                              name=f"mth{mt}{ih}", bufs=1)
                            thi = nc.scalar.activation(th_t[:],
                                                       sp_ts[mt, ih][:],
                                                       AF.Tanh)
                            for e in pre_tanh:
                                add_dep_helper(thi.ins, e.ins, sync=False,
                                               reason="batch act tables")
                            th_ts[mt, ih] = th_t
                    for ih in range(2):
                        sl = bass.ts(ih, 1024)
                        for mt in range(2):
                            nc.vector.scalar_tensor_tensor(
                                h_sb[:, mt, sl], pso[mt, ih][:],
                                b1c[:, mt:mt + 1], th_ts[mt, ih][:],
                                op0=OP.add, op1=OP.mult)
                        for mt in range(2):
                            ps = pso[mt, ih]
                            for q2 in range(2):
                                nc.tensor.matmul(
                                    ps[:, bass.ts(q2, 512)],
                                    w2t[:, :, bass.ts(mt, 128)],
                                    h_sb[:, :, ih * 1024 + q2 * 512:
                                         ih * 1024 + (q2 + 1) * 512],
                                    start=True, stop=True, perf_mode=DR,
                                    skip_group_check=True)
                            y_sb = ypool.tile([128, 1024], F32)
                            nc.vector.scalar_tensor_tensor(
                                y_sb[:], ps[:], b2c[:, mt:mt + 1],
                                xf_sb[:, mt * IS + ih * 1024:
                                      mt * IS + (ih + 1) * 1024],
                                op0=OP.add, op1=OP.add)
                            eng = nc.sync if mt == 0 else nc.scalar
                            eng.dma_start(y_d[bass.ts(mt, 128), sl], y_sb[:])
                vtpool.release()
    nc.finalize()
    return nc


def _pair_halves(w):
    """[256, M] fp32 -> [128, 2, M]: [p, kt, m] = w[kt*128+p, m]."""
    k, m = w.shape
    assert k == 256
    return np.ascontiguousarray(w.reshape(2, 128, m).transpose(1, 0, 2))


def make_in_maps(x, WQ, bQ, WK, bK, WV, bV, PE, W1, b1, W2, b2, n_cores=N_CORES):
    x = np.asarray(x, dtype=np.float32)
    xf3 = np.ascontiguousarray(x.reshape(B, C, N))
    pef = np.asarray(PE, dtype=np.float32).reshape(QK, N)
    pe1 = (pef + np.asarray(bK, np.float32)[:, None]).astype(bf16)
    pe1q_full = (pef + np.asarray(bQ, np.float32)[:, None]).astype(bf16)

    wqk = np.concatenate([
        _pair_halves(np.asarray(WQ, np.float32).T),
        _pair_halves(np.asarray(WK, np.float32).T),
    ], axis=2).astype(f8)
    wmlp = np.concatenate([
        _pair_halves(np.asarray(WV, np.float32).T * VS),
        _pair_halves(np.asarray(W1, np.float32).T),
        _pair_halves(np.asarray(W2, np.float32).T),
    ], axis=2).astype(f8)
    bvb = np.ascontiguousarray(np.broadcast_to(
        (np.asarray(bV, np.float32) * VS)[None, None, :],
        (128, 8, 256)).astype(bf16))
    bcols = np.concatenate([
        np.asarray(b1, np.float32).reshape(2, 128).T,
        np.asarray(b2, np.float32).reshape(2, 128).T,
    ], axis=1)

    shared = {
        "pe1": np.ascontiguousarray(pe1),
        "wqk": np.ascontiguousarray(wqk),
        "wmlp": np.ascontiguousarray(wmlp),
        "bvb": bvb,
        "bcols": np.ascontiguousarray(bcols),
    }
    in_maps = []
    for core in range(n_cores):
        s, h = core // 2, core % 2
        isl = slice(h * IS, (h + 1) * IS)
        xb3 = _pair_halves(xf3[s]).astype(f8)          # [128, 2, N]
        m = dict(shared)
        m["xb"] = np.ascontiguousarray(xb3)
        m["xq"] = np.ascontiguousarray(xb3[:, :, isl])
        m["xf"] = np.ascontiguousarray(
            np.concatenate([xf3[s][:128, isl], xf3[s][128:, isl]], axis=1))
        m["pe1q"] = np.ascontiguousarray(pe1q_full[:, isl])
        in_maps.append(m)
    return in_maps


def assemble_output(results, n_cores=N_CORES):
    y = np.empty((B, C, N), dtype=np.float32)
    for s in range(B):
        y[s][:, :IS] = results[2 * s]["y"]
        y[s][:, IS:] = results[2 * s + 1]["y"]
    return y.reshape(B, C, H, W)


_PROG = None


def kernel(**inputs) -> np.ndarray:
    global _PROG
    from concourse.bass_utils import run_bass_kernel_spmd
    if _PROG is None:
        _PROG = build_program(N_CORES)
    in_maps = make_in_maps(**inputs)
    res = run_bass_kernel_spmd(_PROG, in_maps, core_ids=list(range(N_CORES)))
    return assemble_output(res.results)


# revision 36
# speedup vs baseline: 1.1852x; 1.1852x over previous
"""AttentionHead kernel for 8 Trainium2 NeuronCores — fp8/DoubleRow design.

Problem (per sample, B=4): x:[256,64,64] -> q/k/v 1x1-conv projections
(+positional encoding on q,k), S = q^T k / 8, softmax over the QUERY axis,
out = attn @ v, then 1x1-conv MLP with Mish + residual.

Sharding: 2 cores per sample, split over the query axis i (2048 queries each).
softmax normalizes over i, so the per-key denominator den[j] = sum_i exp(S[i,j])
needs one tiny cross-pair AllReduce (3 chunks, latency hidden behind the
exp stream / attn@v); den folds into v, everything else is local.

Design notes (vs the 194us bf16 baseline):
- The critical path is ScalarE's exp stream: 32 x [128,2048] tiles at ~2.1us
  each, running gapless on fully double-buffered PSUM (two alternating 4-bank
  pools). S matmuls (bf16, ~0.43us/tile at the 1.2GHz mid p-state) hide
  underneath, as do the v-projection "mega tiles" (8 j-tiles of fp8-DoubleRow
  matmul + one DVE copy) that borrow stream PSUM slots.
- attn@v, q/k/v projections and both MLP matmuls run fp8e4 DoubleRow (two
  128-deep k-tiles per matmul at 0.5 cycles/row), pairing adjacent j-tiles /
  channel halves via 3D APs over the natural layouts — no repacking.
  Measured end-to-end rel err ~7e-4 (residual + MLP attenuate the
  attention-path quantization).
- attn is stored as fp8 exp(S/8 - 2); the -2 keeps e4m3 in range and cancels
  in the softmax. ScalarE writes it directly from PSUM with the denominator
  accumulated for free. v is pre-scaled by 32 (folded into Wv/bv on host) so
  vts = v*32/den fits fp8; the attn@v PSUM->SBUF copy divides by 32.
- Mish: no Mish/Softplus LUT exists on this stack, so h = (h1)*tanh(sp),
  sp = Ln(1 + e^{h1}) — Exp and Ln live in the same activation table set as
  the attention exps (zero switches); the Tanh batch costs the single table
  switch of the whole kernel. GpSimd is avoided for elementwise work (its
  software ops measure ~12ns/elem) and cannot touch PSUM.
- Input DMAs: the 4MB fp32 residual (xf) is deferred until mid-stream so the
  q/k-path tensors own the DMA engines at the start; a 1-element Exp preloads
  the activation table during the initial DMA wait.
"""

import numpy as np
import ml_dtypes

import concourse.bass as bass
import concourse.bacc as bacc
import concourse.mybir as mybir
import concourse.tile as tile
from concourse.tile_rust import add_dep_helper

BF16 = mybir.dt.bfloat16
F8 = mybir.dt.float8e4
F32 = mybir.dt.float32
AF = mybir.ActivationFunctionType
OP = mybir.AluOpType
DR = mybir.MatmulPerfMode.DoubleRow
bf16 = ml_dtypes.bfloat16
f8 = ml_dtypes.float8_e4m3

B, C, H, W = 4, 256, 64, 64
N = H * W            # 4096 pixels
QK = 64
IS = N // 2          # 2048 queries per core
NJT = N // 128       # 32 key tiles
NIB = IS // 512      # 4 i-blocks
NBOOT = 2            # S tiles computed during the projection boot
CH_A, CH_B = 16, 28  # den chunk boundaries: [0,16) [16,28) [28,32)
VS = 32.0            # v scale (folded into Wv/bv on host)
EXP_BIAS = -2.0
N_CORES = 8
REPLICA_GROUPS = [[0, 1], [2, 3], [4, 5], [6, 7]]


def build_program(n_cores: int = N_CORES) -> bass.Bass:
    nc = bacc.Bacc(
        "TRN2",
        target_bir_lowering=False,
        debug=False,
        enable_asserts=False,
        num_devices=n_cores,
    )

    # Per-core inputs. xq/xb hold the two 128-row channel halves as dim 1
    # (the DoubleRow pair axis for the channel contraction).
    xq_d = nc.dram_tensor("xq", [128, 2, IS], F8, kind="ExternalInput").ap()
    xb_d = nc.dram_tensor("xb", [128, 2, N], F8, kind="ExternalInput").ap()
    xf_d = nc.dram_tensor("xf", [128, 2 * IS], F32, kind="ExternalInput").ap()
    pe1q_d = nc.dram_tensor("pe1q", [QK, IS], BF16, kind="ExternalInput").ap()
    # Shared weights (same on all cores).
    pe1_d = nc.dram_tensor("pe1", [QK, N], BF16, kind="ExternalInput").ap()
    wqk_d = nc.dram_tensor("wqk", [128, 2, 128], F8, kind="ExternalInput").ap()
    # wmlp = wvt*VS | w1t | w2t along the last axis
    wmlp_d = nc.dram_tensor("wmlp", [128, 2, 768], F8, kind="ExternalInput").ap()
    bvb_d = nc.dram_tensor("bvb", [128, 8, 256], BF16, kind="ExternalInput").ap()
    bcols_d = nc.dram_tensor("bcols", [128, 4], F32, kind="ExternalInput").ap()

    y_d = nc.dram_tensor("y", [C, IS], F32, kind="ExternalOutput").ap()

    with tile.TileContext(nc) as tc:
        with (
            tc.tile_pool(name="const", bufs=1) as cpool,
            tc.tile_pool(name="qk", bufs=1) as qkpool,
            tc.tile_pool(name="den", bufs=1) as denpool,
            tc.tile_pool(name="io", bufs=1) as iopool,
            tc.tile_pool(name="dram", bufs=1, space="DRAM") as dram,
        ):
            ebias_sb = cpool.tile([128, 1], F32)
            nc.gpsimd.memset(ebias_sb[:], EXP_BIAS)
            # Preload the Exp activation table while input DMAs run.
            warm_sb = cpool.tile([128, 1], BF16)
            nc.scalar.activation(warm_sb[:], ebias_sb[:], AF.Exp)

            # Critical-path loads, all contiguous per partition: wqk, xq,
            # pe1q, then xb ordered so k block 0 (j 0..2047) lands first.
            wqk_sb = cpool.tile([128, 2, 128], F8)
            nc.sync.dma_start(wqk_sb[:], wqk_d[:])
            xq_sb = iopool.tile([128, 2, IS], F8)
            for kt in range(2):
                nc.sync.dma_start(xq_sb[:, kt, :], xq_d[:, kt, :])
            pe1q_sb = iopool.tile([QK, IS], BF16)
            nc.scalar.dma_start(pe1q_sb[:], pe1q_d[:])
            xb_sb = iopool.tile([128, 2, N], F8)
            for hh in range(2):
                for kt in range(2):
                    nc.sync.dma_start(xb_sb[:, kt, bass.ts(hh, N // 2)],
                                      xb_d[:, kt, bass.ts(hh, N // 2)])
            pe1_sb = iopool.tile([QK, N], BF16)
            pe1_is = []
            for hh in range(2):
                pe1_is.append(
                    nc.scalar.dma_start(pe1_sb[:, bass.ts(hh, N // 2)],
                                        pe1_d[:, bass.ts(hh, N // 2)]))

            wmlp_sb = cpool.tile([128, 2, 768], F8)
            bvb_sb = cpool.tile([128, 8, 256], BF16)
            bcols_sb = cpool.tile([128, 4], F32)
            nc.sync.dma_start(wmlp_sb[:], wmlp_d[:])
            nc.sync.dma_start(bvb_sb[:], bvb_d[:])
            nc.sync.dma_start(bcols_sb[:], bcols_d[:])
            xf_sb = iopool.tile([128, 2 * IS], F32)  # DMA deferred, see below

            wvt = wmlp_sb[:, :, 0:256]
            w1t = wmlp_sb[:, :, 256:512]
            w2t = wmlp_sb[:, :, 512:768]
            b1c = bcols_sb[:, 0:2]
            b2c = bcols_sb[:, 2:4]

            q_sb = qkpool.tile([QK, IS], BF16)
            k_sb = qkpool.tile([QK, N], BF16)
            vtpool = tc.alloc_tile_pool(name="vt", bufs=1, side="right")
            vt_sb = vtpool.tile([128, NJT, 256], BF16)
            vts_sb = qkpool.tile([128, NJT, 256], F8)
            den_sb = denpool.tile([128, NJT], F32)
            den_h = denpool.tile([128, 2], F32)
            dsum_sb = denpool.tile([128, NJT], F32)
            rden_sb = denpool.tile([128, NJT], F32)

            with tc.tile_pool(name="attn", bufs=1) as apool:
                attn_sb = apool.tile([128, NJT, IS], F8)   # 8 MiB

                with tc.tile_pool(name="psB", bufs=1, space="PSUM") as psB:
                    exp_is = {}

                    def s_tile(t, pool):
                        ps = pool.tile([128, IS], F32, name="pss")
                        for ib in range(NIB):
                            nc.tensor.matmul(ps[:, bass.ts(ib, 512)],
                                             k_sb[:, bass.ts(t, 128)],
                                             q_sb[:, bass.ts(ib, 512)],
                                             start=True, stop=True)
                        exp_is[t] = nc.scalar.activation(
                            attn_sb[:, t, :], ps[:], AF.Exp, scale=0.125,
                            bias=ebias_sb[:], accum_out=den_sb[:, t:t + 1])

                    def v_mega(vm, pool):
                        ps = pool.tile([128, 4, 256], F32, name="pss")
                        for g in range(4):
                            jt = 4 * vm + g
                            nc.tensor.matmul(ps[:, g, :],
                                             xb_sb[:, :, bass.ts(jt, 128)],
                                             wvt, start=True, stop=True,
                                             perf_mode=DR)
                        nc.vector.tensor_add(vt_sb[:, 4 * vm:4 * vm + 4, :],
                                             ps[:], bvb_sb[:, 0:4, :])

                    # ---- boot: q/k projections + S tiles 0..2 ----
                    with (
                        tc.tile_pool(name="psQ", bufs=2, space="PSUM") as psQ,
                        tc.tile_pool(name="psK", bufs=2, space="PSUM") as psK,
                    ):
                        def q_proj(ib):
                            sl = bass.ts(ib, 512)
                            ps = psQ.tile([QK, 512], F32, name="psq")
                            nc.tensor.matmul(ps[:], wqk_sb[:, :, 0:QK],
                                             xq_sb[:, :, sl],
                                             start=True, stop=True,
                                             perf_mode=DR)
                            nc.vector.tensor_add(q_sb[:, sl], ps[:],
                                                 pe1q_sb[:, sl])

                        def k_proj(jb):
                            sl = bass.ts(jb, 512)
                            ps = psK.tile([QK, 512], F32, name="psk")
                            nc.tensor.matmul(ps[:], wqk_sb[:, :, QK:2 * QK],
                                             xb_sb[:, :, sl],
                                             start=True, stop=True,
                                             perf_mode=DR)
                            nc.vector.tensor_add(k_sb[:, sl], ps[:],
                                                 pe1_sb[:, sl])

                        # tile 0 in two i-halves so the exp stream starts as
                        # soon as q ib0/ib1 + the first k block are in
                        q_proj(0)
                        q_proj(1)
                        k_proj(0)
                        ps0 = psB.tile([128, IS], F32, name="pss")
                        for ib in range(2):
                            nc.tensor.matmul(ps0[:, bass.ts(ib, 512)],
                                             k_sb[:, 0:128],
                                             q_sb[:, bass.ts(ib, 512)],
                                             start=True, stop=True)
                        e0a = nc.scalar.activation(
                            attn_sb[:, 0, 0:1024], ps0[:, 0:1024],
                            AF.Exp, scale=0.125, bias=ebias_sb[:],
                            accum_out=den_h[:, 0:1])
                        q_proj(2)
                        q_proj(3)
                        for ib in range(2, 4):
                            nc.tensor.matmul(ps0[:, bass.ts(ib, 512)],
                                             k_sb[:, 0:128],
                                             q_sb[:, bass.ts(ib, 512)],
                                             start=True, stop=True)
                        nc.scalar.activation(
                            attn_sb[:, 0, 1024:2048], ps0[:, 1024:2048],
                            AF.Exp, scale=0.125, bias=ebias_sb[:],
                            accum_out=den_h[:, 1:2])
                        k_proj(1)
                        s_tile(1, psB)
                        for jb in range(2, 8):
                            k_proj(jb)
                        nc.vector.tensor_add(den_sb[:, 0:1], den_h[:, 0:1],
                                             den_h[:, 1:2])

                    # ---- den AllReduce within the core pair ----
                    def exchange(lo, hi, tag):
                        w = hi - lo
                        den_in = dram.tile([128, w], F32, name=f"den_in{tag}")
                        den_out = dram.tile([128, w], F32, name=f"den_out{tag}")
                        nc.sync.dma_start(den_in[:], den_sb[:, lo:hi])
                        nc.gpsimd.collective_compute(
                            "AllReduce", OP.add,
                            replica_groups=REPLICA_GROUPS,
                            ins=[den_in.opt()], outs=[den_out.opt()],
                        )
                        nc.sync.dma_start(dsum_sb[:, lo:hi], den_out[:])
                        nc.vector.reciprocal(rden_sb[:, lo:hi],
                                             dsum_sb[:, lo:hi])
                        for jt in range(lo, hi):
                            nc.vector.tensor_scalar_mul(vts_sb[:, jt, :],
                                                        vt_sb[:, jt, :],
                                                        rden_sb[:, jt:jt + 1])

                    # ---- main stream: S tiles 3..31 ping-ponging between the
                    # boot pool and a second 4-bank pool; v-projection mega
                    # tiles borrow stream slots ----
                    VM_AFTER = {4: 0, 7: 1, 10: 2, 13: 3, 16: 4, 19: 5, 22: 6, 25: 7}
                    with tc.tile_pool(name="psS", bufs=1, space="PSUM") as psS:
                        pools = [psS, psB]
                        pi = 0
                        for t in range(NBOOT, NJT):
                            s_tile(t, pools[pi % 2]); pi += 1
                            if t in VM_AFTER:
                                # reuse the slot of the tile just exp'd; the
                                # S-tile alternation parity is unaffected
                                v_mega(VM_AFTER[t], pools[(pi - 1) % 2])
                            if t == CH_A - 1:
                                exchange(0, CH_A, "A")
                                for hh in range(4):
                                    nc.sync.dma_start(
                                        xf_sb[:, bass.ts(hh, IS // 2)],
                                        xf_d[:, bass.ts(hh, IS // 2)])
                            elif t == CH_B - 1:
                                exchange(CH_A, CH_B, "B")
                        exchange(CH_B, NJT, "C")

                # ---- attn@v + MLP, [128,1024] grain (ih halves) ----
                with (
                    tc.tile_pool(name="oh", bufs=1) as ohpool,
                    tc.tile_pool(name="mtmp", bufs=2) as mpool,
                    tc.tile_pool(name="y", bufs=2) as ypool,
                    tc.tile_pool(name="psO", bufs=1, space="PSUM") as psO,
                ):
                    out_sb = ohpool.tile([128, 2, IS], F8)
                    h_sb = ohpool.tile([128, 2, IS], F8)
                    pso = {}
                    for mt in range(2):
                        for ih in range(2):
                            pso[mt, ih] = psO.tile([128, 1024], F32,
                                                   name=f"pso{mt}{ih}")

                    def av_pairs(mt, ih, plo, phi):
                        for p in range(plo, phi):
                            for q2 in range(2):
                                nc.tensor.matmul(
                                    pso[mt, ih][:, bass.ts(q2, 512)],
                                    vts_sb[:, 2 * p:2 * p + 2,
                                           bass.ts(mt, 128)],
                                    attn_sb[:, 2 * p:2 * p + 2,
                                            ih * 1024 + q2 * 512:
                                            ih * 1024 + (q2 + 1) * 512],
                                    start=(p == 0), stop=(p == NJT // 2 - 1),
                                    perf_mode=DR, skip_group_check=True)

                    sp_ts = {}
                    pre_tanh = []

                    t_ts = {}
                    exp_mis = []

                    def mlp_front(ih):
                        # out copies (both mt halves), then W1 and the mish
                        # exp for the two [128,1024] blocks of this ih (runs
                        # while later AV blocks matmul); the Ln/Tanh batches
                        # come after all fronts
                        sl = bass.ts(ih, 1024)
                        for mt in range(2):
                            ci = nc.scalar.activation(out_sb[:, mt, sl],
                                                      pso[mt, ih][:], AF.Copy,
                                                      scale=1.0 / VS)
                            pre_tanh.append(ci)
                        for mt in range(2):
                            ps = pso[mt, ih]
                            for q2 in range(2):
                                nc.tensor.matmul(
                                    ps[:, bass.ts(q2, 512)],
                                    w1t[:, :, bass.ts(mt, 128)],
                                    out_sb[:, :, ih * 1024 + q2 * 512:
                                           ih * 1024 + (q2 + 1) * 512],
                                    start=True, stop=True, perf_mode=DR,
                                    skip_group_check=True)
                            t_t = mpool.tile([128, 1024], BF16,
                                             name=f"mt{mt}{ih}", bufs=1)
                            ti = nc.scalar.activation(t_t[:], ps[:], AF.Exp,
                                                      bias=b1c[:, mt:mt + 1])
                            pre_tanh.append(ti)
                            exp_mis.append(ti)
                            t_ts[mt, ih] = t_t

                    # W1's out_sb input needs both mt halves of its ih, so
                    # blocks complete in (ih-major, mt-minor) order; each
                    # block's MLP front starts while the next blocks' attn@v
                    # matmuls keep the PE busy
                    AB, CE = CH_B // 2, NJT // 2
                    av_pairs(0, 0, 0, AB)
                    av_pairs(1, 0, 0, AB)
                    av_pairs(0, 1, 0, AB)
                    av_pairs(0, 0, AB, CE)
                    av_pairs(1, 0, AB, CE)
                    mlp_front(0)
                    av_pairs(1, 1, 0, AB)
                    av_pairs(0, 1, AB, CE)
                    av_pairs(1, 1, AB, CE)
                    mlp_front(1)

                    # ln batch (all exps first, one load), then the tanh
                    # batch (second load)
                    for ih in range(2):
                        for mt in range(2):
                            sp_t = mpool.tile([128, 1024], BF16,
                                              name=f"msp{mt}{ih}", bufs=1)
                            li = nc.scalar.activation(sp_t[:],
                                                      t_ts[mt, ih][:],
                                                      AF.Ln, bias=1.0)
                            for e in exp_mis:
                                add_dep_helper(li.ins, e.ins, sync=False,
                                               reason="batch act tables")
                            pre_tanh.append(li)
                            sp_ts[mt, ih] = sp_t

                    # tanh batch (the kernel's one table switch); pin every
                    # tanh after every exp/ln/copy so the scheduler can't
                    # ping-pong the act tables
                    th_ts = {}
                    for ih in range(2):
                        for mt in range(2):
                            th_t = mpool.tile([128, 1024], BF16,
                                              name=f"mth{mt}{ih}", bufs=1)
                            thi = nc.scalar.activation(th_t[:],
                                                       sp_ts[mt, ih][:],
                                                       AF.Tanh)
                            for e in pre_tanh:
                                add_dep_helper(thi.ins, e.ins, sync=False,
                                               reason="batch act tables")
                            th_ts[mt, ih] = th_t
                    for ih in range(2):
                        sl = bass.ts(ih, 1024)
                        for mt in range(2):
                            nc.vector.scalar_tensor_tensor(
                                h_sb[:, mt, sl], pso[mt, ih][:],
                                b1c[:, mt:mt + 1], th_ts[mt, ih][:],
                                op0=OP.add, op1=OP.mult)
                        for mt in range(2):
                            ps = pso[mt, ih]
                            for q2 in range(2):
                                nc.tensor.matmul(
                                    ps[:, bass.ts(q2, 512)],
                                    w2t[:, :, bass.ts(mt, 128)],
                                    h_sb[:, :, ih * 1024 + q2 * 512:
                                         ih * 1024 + (q2 + 1) * 512],
                                    start=True, stop=True, perf_mode=DR,
                                    skip_group_check=True)
                            y_sb = ypool.tile([128, 1024], F32)
                            nc.vector.scalar_tensor_tensor(
                                y_sb[:], ps[:], b2c[:, mt:mt + 1],
                                xf_sb[:, mt * IS + ih * 1024:
                                      mt * IS + (ih + 1) * 1024],
                                op0=OP.add, op1=OP.add)
                            eng = nc.sync if mt == 0 else nc.scalar
                            eng.dma_start(y_d[bass.ts(mt, 128), sl], y_sb[:])
                vtpool.release()
    nc.finalize()
    return nc


def _pair_halves(w):
    """[256, M] fp32 -> [128, 2, M]: [p, kt, m] = w[kt*128+p, m]."""
    k, m = w.shape
    assert k == 256
    return np.ascontiguousarray(w.reshape(2, 128, m).transpose(1, 0, 2))


def make_in_maps(x, WQ, bQ, WK, bK, WV, bV, PE, W1, b1, W2, b2, n_cores=N_CORES):
    x = np.asarray(x, dtype=np.float32)
    xf3 = np.ascontiguousarray(x.reshape(B, C, N))
    pef = np.asarray(PE, dtype=np.float32).reshape(QK, N)
    pe1 = (pef + np.asarray(bK, np.float32)[:, None]).astype(bf16)
    pe1q_full = (pef + np.asarray(bQ, np.float32)[:, None]).astype(bf16)

    wqk = np.concatenate([
        _pair_halves(np.asarray(WQ, np.float32).T),
        _pair_halves(np.asarray(WK, np.float32).T),
    ], axis=2).astype(f8)
    wmlp = np.concatenate([
        _pair_halves(np.asarray(WV, np.float32).T * VS),
        _pair_halves(np.asarray(W1, np.float32).T),
        _pair_halves(np.asarray(W2, np.float32).T),
    ], axis=2).astype(f8)
    bvb = np.ascontiguousarray(np.broadcast_to(
        (np.asarray(bV, np.float32) * VS)[None, None, :],
        (128, 8, 256)).astype(bf16))
    bcols = np.concatenate([
        np.asarray(b1, np.float32).reshape(2, 128).T,
        np.asarray(b2, np.float32).reshape(2, 128).T,
    ], axis=1)

    shared = {
        "pe1": np.ascontiguousarray(pe1),
        "wqk": np.ascontiguousarray(wqk),
        "wmlp": np.ascontiguousarray(wmlp),
        "bvb": bvb,
        "bcols": np.ascontiguousarray(bcols),
    }
    in_maps = []
    for core in range(n_cores):
        s, h = core // 2, core % 2
        isl = slice(h * IS, (h + 1) * IS)
        xb3 = _pair_halves(xf3[s]).astype(f8)          # [128, 2, N]
        m = dict(shared)
        m["xb"] = np.ascontiguousarray(xb3)
        m["xq"] = np.ascontiguousarray(xb3[:, :, isl])
        m["xf"] = np.ascontiguousarray(
            np.concatenate([xf3[s][:128, isl], xf3[s][128:, isl]], axis=1))
        m["pe1q"] = np.ascontiguousarray(pe1q_full[:, isl])
        in_maps.append(m)
    return in_maps


def assemble_output(results, n_cores=N_CORES):
    y = np.empty((B, C, N), dtype=np.float32)
    for s in range(B):
        y[s][:, :IS] = results[2 * s]["y"]
        y[s][:, IS:] = results[2 * s + 1]["y"]
    return y.reshape(B, C, H, W)


_PROG = None


def kernel(**inputs) -> np.ndarray:
    global _PROG
    from concourse.bass_utils import run_bass_kernel_spmd
    if _PROG is None:
        _PROG = build_program(N_CORES)
    in_maps = make_in_maps(**inputs)
    res = run_bass_kernel_spmd(_PROG, in_maps, core_ids=list(range(N_CORES)))
    return assemble_output(res.results)
